# revision 8
# baseline (speedup 1.0000x reference)
"""Trainium2 Bass kernel for the heterogeneous GNN (GAT + SAGE, 2 layers).

Strategy: destination-node sharding across 8 cores (papers 12500/core,
authors 6250/core). Each layer:
  1. Per-core dense pass over the LOCAL node shard computes projected
     gather-tables:  F_a  = [h_a @ Wsrc_w | al_s_w]        (132 f32/row)
                     F_pg = [h_p @ Wsrc_auth | al_s_auth]  (132)
                     F_ps = [h_p @ Wl_cites]               (128)
     plus local attention dst-terms al_d (h @ (Wdst·adst)).
  2. AllGather the three tables (all cores get full copies).
  3. Edge phase: edges are pre-sorted by destination on the host and cut
     into 128-edge chunks per 128-destination tile. Per chunk: one
     indirect-DMA row gather from the F table, a selection matrix
     S[e,j] = (dst_rel[e] == j) built on the vector engine, softmax
     numerators exp(leakyrelu(al_s+al_d)) (max-shift dropped - softmax is
     shift invariant), messages scaled per head, then ONE matmul
     S.T @ msg accumulating into PSUM (plus a 4-wide matmul accumulating
     softmax denominators into the same PSUM tile's tail columns).
  4. Softmax division per destination after aggregation, SAGE mean via
     host-precomputed reciprocal counts, + h_dst @ Wr, LayerNorm, ReLU,
     residual - all local to the shard (h kept SBUF-resident).
Weight tensors are replicated; biases/ln params that are zero/one in the
given inputs elide their ops at program-build time.
"""
import sys

if "/opt/trn_rl_repo" not in sys.path:
    sys.path.insert(0, "/opt/trn_rl_repo")

import numpy as np

NCORES = 8
NA, NP_ = 50000, 100000
H, HEADS, CH = 128, 4, 32
IN_A, IN_P = 128, 256
LN_EPS = 1e-5
L = 2
P = 128
PSH, ASH = NP_ // NCORES, NA // NCORES          # 12500, 6250
PT, AT = (PSH + P - 1) // P, (ASH + P - 1) // P  # 98, 49


def _prep_edges(src, dst, shard, n_tiles, need_loc):
    """Sort edges by dst, shard by dst owner, cut into per-tile 128-edge
    chunks padded to a chunk count that is uniform ACROSS CORES (the SPMD
    program is shared). Returns (K[t] per tile, per-core dicts of
    [128, Q] arrays: src ids, dst_rel f32, dst local idx)."""
    src = np.asarray(src).astype(np.int64)
    dst = np.asarray(dst).astype(np.int64)
    per_core = []
    cnts = np.zeros((NCORES, n_tiles), np.int64)
    for r in range(NCORES):
        lo, hi = r * shard, (r + 1) * shard
        sel = (dst >= lo) & (dst < hi)
        s, d = src[sel], dst[sel] - lo
        o = np.argsort(d, kind="stable")
        s, d = s[o], d[o]
        t = d >> 7
        cnts[r] = np.bincount(t, minlength=n_tiles)
        per_core.append((s, d))
    K = ((cnts + P - 1) // P).max(axis=0)          # chunks per tile
    off = np.concatenate([[0], np.cumsum(K)]).astype(np.int64)
    Q = int(off[-1])
    out = []
    for r in range(NCORES):
        s, d = per_core[r]
        src_a = np.zeros((P, Q), np.int32)
        rel_a = np.full((P, Q), -1.0, np.float32)
        loc_a = np.zeros((P, Q), np.int32)
        bounds = np.concatenate([[0], np.cumsum(cnts[r])])
        for t in range(n_tiles):
            b0, b1 = bounds[t], bounds[t + 1]
            n = b1 - b0
            if n == 0:
                continue
            j = np.arange(n)
            col = off[t] + (j >> 7)
            row = j & 127
            src_a[row, col] = s[b0:b1]
            rel_a[row, col] = (d[b0:b1] - t * P).astype(np.float32)
            loc_a[row, col] = d[b0:b1]
        ent = {"src": src_a, "rel": rel_a}
        if need_loc:
            ent["loc"] = loc_a
        out.append(ent)
    return K.astype(np.int64), off, Q, out


def _build_and_run(inp):
    import concourse.bass as bass
    import concourse.mybir as mybir
    import concourse.tile as tile
    from concourse import bacc
    from concourse.masks import make_identity
    from concourse.bass_utils import run_bass_kernel_spmd

    f32, i32 = mybir.dt.float32, mybir.dt.int32

    g = lambda k: np.asarray(inp[k], np.float32)
    gi = lambda k: np.asarray(inp[k], np.int32)

    # ---------- host prep: edges ----------
    Kw, offw, Qw, ew = _prep_edges(gi("writes_src"), gi("writes_dst"), PSH, PT, True)
    Kc, offc, Qc, ec = _prep_edges(gi("cites_src"), gi("cites_dst"), PSH, PT, False)
    Ka, offa, Qa, ea = _prep_edges(gi("auth_src"), gi("auth_dst"), ASH, AT, True)

    # SAGE reciprocal counts per destination paper, tile-column layout
    cnt = np.bincount(gi("cites_dst"), minlength=NP_).astype(np.float32)
    rcp = 1.0 / np.maximum(cnt, 1.0)
    rcp_T = np.ones((P, PT * NCORES), np.float32)
    for r in range(NCORES):
        blk = rcp[r * PSH:(r + 1) * PSH]
        blk = np.pad(blk, (0, PT * P - PSH), constant_values=1.0)
        rcp_T[:, r * PT:(r + 1) * PT] = blk.reshape(PT, P).T

    # ---------- host prep: weights ----------
    iota_np = np.tile(np.arange(P, dtype=np.float32), (P, 1))
    wdict = {"iota": iota_np,
             "w_emb_a": g("emb_author_W"), "w_emb_p": g("emb_paper_W"),
             "w_out_a": g("out_author_W"), "w_out_p": g("out_paper_W")}
    for l in range(L):
        wdict[f"wsw{l}"] = g("gat_writes_Wsrc")[l]
        wdict[f"wsa{l}"] = g("gat_auth_Wsrc")[l]
        wdict[f"wl{l}"] = g("sage_cites_Wl")[l]
        wdict[f"wr{l}"] = g("sage_cites_Wr")[l]
        for nm, wk, ak in (("uw", "gat_writes_Wdst", "gat_writes_adst"),
                           ("ua", "gat_auth_Wdst", "gat_auth_adst")):
            W, a = g(wk)[l], g(ak)[l]
            wdict[f"{nm}{l}"] = (W.reshape(H, HEADS, CH) * a[None]).sum(-1)  # [H,4]
        wdict[f"asw{l}"] = np.tile(g("gat_writes_asrc")[l].reshape(1, H), (P, 1))
        wdict[f"asa{l}"] = np.tile(g("gat_auth_asrc")[l].reshape(1, H), (P, 1))

    # optional bias / ln tiles (elided when trivial)
    def rep(v):
        return np.tile(np.asarray(v, np.float32).reshape(1, H), (P, 1))
    nz = lambda v: not np.all(np.asarray(v) == 0.0)
    none1 = lambda v: not np.all(np.asarray(v) == 1.0)
    emb_a_b, emb_p_b = nz(inp["emb_author_b"]), nz(inp["emb_paper_b"])
    out_a_b, out_p_b = nz(inp["out_author_b"]), nz(inp["out_paper_b"])
    bias_p = [g("gat_writes_b")[l] + g("sage_cites_bl")[l] + g("sage_cites_br")[l]
              for l in range(L)]
    bias_a = [g("gat_auth_b")[l] for l in range(L)]
    use_bias_p = [nz(b) for b in bias_p]
    use_bias_a = [nz(b) for b in bias_a]
    use_ln_g = [[none1(g("ln_paper_g")[l]), none1(g("ln_author_g")[l])] for l in range(L)]
    use_ln_b = [[nz(g("ln_paper_b")[l]), nz(g("ln_author_b")[l])] for l in range(L)]
    for l in range(L):
        if emb_a_b: wdict["emb_a_b"] = rep(inp["emb_author_b"])
        if emb_p_b: wdict["emb_p_b"] = rep(inp["emb_paper_b"])
        if out_a_b: wdict["out_a_b"] = rep(inp["out_author_b"])
        if out_p_b: wdict["out_p_b"] = rep(inp["out_paper_b"])
        if use_bias_p[l]: wdict[f"bias_p{l}"] = rep(bias_p[l])
        if use_bias_a[l]: wdict[f"bias_a{l}"] = rep(bias_a[l])
        if use_ln_g[l][0]: wdict[f"lng_p{l}"] = rep(g("ln_paper_g")[l])
        if use_ln_g[l][1]: wdict[f"lng_a{l}"] = rep(g("ln_author_g")[l])
        if use_ln_b[l][0]: wdict[f"lnb_p{l}"] = rep(g("ln_paper_b")[l])
        if use_ln_b[l][1]: wdict[f"lnb_a{l}"] = rep(g("ln_author_b")[l])

    # per-core inputs
    xa = g("x_author"); xp = g("x_paper")
    in_maps = []
    for r in range(NCORES):
        m = dict(wdict)
        m["x_a"] = np.pad(xa[r * ASH:(r + 1) * ASH], ((0, AT * P - ASH), (0, 0)))
        m["x_p"] = np.pad(xp[r * PSH:(r + 1) * PSH], ((0, PT * P - PSH), (0, 0)))
        m["src_w"], m["rel_w"], m["loc_w"] = ew[r]["src"], ew[r]["rel"], ew[r]["loc"]
        m["src_c"], m["rel_c"] = ec[r]["src"], ec[r]["rel"]
        m["src_a"], m["rel_a"], m["loc_a"] = ea[r]["src"], ea[r]["rel"], ea[r]["loc"]
        m["rcp_c"] = np.ascontiguousarray(rcp_T[:, r * PT:(r + 1) * PT])
        in_maps.append(m)

    # ---------- program ----------
    nc = bacc.Bacc("TRN2", target_bir_lowering=False, debug=False,
                   num_devices=NCORES)
    ein = lambda n, s, dt=f32: nc.dram_tensor(n, s, dt, kind="ExternalInput").ap()
    eout = lambda n, s: nc.dram_tensor(n, s, f32, kind="ExternalOutput").ap()

    d_in = {k: ein(k, list(v.shape), i32 if v.dtype == np.int32 else f32)
            for k, v in in_maps[0].items()}
    o_a = eout("o_a", [ASH, H])
    o_p = eout("o_p", [PSH, H])
    import os as _os
    DBG = int(_os.environ.get("KDBG", "0"))
    if DBG:
        d_emb_p = eout("d_emb_p", [PT * P, H])
        d_fa = eout("d_fa", [NA, 132])
        d_fpg = eout("d_fpg", [NP_, 132])
        d_fps = eout("d_fps", [NP_, H])
        d_hp1 = eout("d_hp1", [PT * P, H])
        d_ha1 = eout("d_ha1", [AT * P, H])
        d_aldp = eout("d_aldp", [PSH, 4])

    fa_in = [nc.dram_tensor(f"fa_in{l}", [ASH, 132], f32).ap() for l in range(L)]
    fpg_in = [nc.dram_tensor(f"fpg_in{l}", [PSH, 132], f32).ap() for l in range(L)]
    fps_in = [nc.dram_tensor(f"fps_in{l}", [PSH, H], f32).ap() for l in range(L)]
    fa_full = [nc.dram_tensor(f"fa_full{l}", [NA, 132], f32,
                              addr_space="Shared").ap() for l in range(L)]
    fpg_full = [nc.dram_tensor(f"fpg_full{l}", [NP_, 132], f32,
                               addr_space="Shared").ap() for l in range(L)]
    fps_full = [nc.dram_tensor(f"fps_full{l}", [NP_, H], f32,
                               addr_space="Shared").ap() for l in range(L)]
    aldp = [nc.dram_tensor(f"aldp{l}", [PSH, 4], f32).ap() for l in range(L)]
    alda = [nc.dram_tensor(f"alda{l}", [ASH, 4], f32).ap() for l in range(L)]

    RG = [list(range(NCORES))]
    AF = mybir.ActivationFunctionType
    OP = mybir.AluOpType
    h4 = lambda ap: ap.rearrange("p (h c) -> p h c", h=HEADS)

    with tile.TileContext(nc) as tc:
        with tc.tile_pool(name="const", bufs=1) as cp, \
             tc.tile_pool(name="meta", bufs=1) as mp, \
             tc.tile_pool(name="work", bufs=3) as wp, \
             tc.tile_pool(name="gat", bufs=4) as gp, \
             tc.tile_pool(name="psA", bufs=2, space="PSUM") as psA, \
             tc.tile_pool(name="psB", bufs=2, space="PSUM") as psB:

            # ---- resident constants ----
            def cload(name, shape=None, dt=f32):
                t = cp.tile(shape or list(in_maps[0][name].shape), dt, tag=name)
                nc.sync.dma_start(t[:], d_in[name][:])
                return t
            ident = cp.tile([P, P], f32, tag="ident")
            make_identity(nc, ident[:])
            eps_t = cp.tile([P, 1], f32, tag="epsc")
            nc.gpsimd.memset(eps_t[:], LN_EPS)
            iota = cload("iota")
            w_emb_a = cload("w_emb_a")
            w_emb_p0 = cp.tile([P, H], f32, tag="wep0")
            w_emb_p1 = cp.tile([P, H], f32, tag="wep1")
            nc.sync.dma_start(w_emb_p0[:], d_in["w_emb_p"][0:P, :])
            nc.sync.dma_start(w_emb_p1[:], d_in["w_emb_p"][P:2 * P, :])
            w_out_a, w_out_p = cload("w_out_a"), cload("w_out_p")
            WS = {k: cload(k) for k in
                  [f"{n}{l}" for l in range(L)
                   for n in ("wsw", "wsa", "wl", "wr", "uw", "ua", "asw", "asa")]}
            OPT = {k: cload(k) for k in wdict if k.startswith(("bias_", "lng_",
                                                               "lnb_", "emb_", "out_"))
                   if k in d_in and k not in ("out_a", "out_p")}
            # edge metadata + counts
            META = {k: mp.tile(list(in_maps[0][k].shape),
                               i32 if in_maps[0][k].dtype == np.int32 else f32,
                               tag=k, name=k)
                    for k in ("src_w", "rel_w", "loc_w", "src_c", "rel_c",
                              "src_a", "rel_a", "loc_a", "rcp_c")}
            for k, t in META.items():
                nc.sync.dma_start(t[:], d_in[k][:])
            # resident node states
            h_p = cp.tile([P, PT * H], f32, tag="h_p")
            h_a = cp.tile([P, AT * H], f32, tag="h_a")

            def transpose_to_sbuf(src_ap, tag):
                tp = psA.tile([P, P], f32, tag="T")
                nc.tensor.transpose(out=tp[:], in_=src_ap, identity=ident[:])
                sb = wp.tile([P, P], f32, tag=tag)
                nc.vector.tensor_copy(sb[:], tp[:])
                return sb

            # ---- embeddings ----
            for t in range(PT):
                xt = wp.tile([P, IN_P], f32, tag="xt")
                nc.sync.dma_start(xt[:], d_in["x_p"][t * P:(t + 1) * P, :])
                tp0 = psA.tile([P, P], f32, tag="T")
                nc.tensor.transpose(out=tp0[:], in_=xt[:, 0:P], identity=ident[:])
                tp1 = psA.tile([P, P], f32, tag="T")
                nc.tensor.transpose(out=tp1[:], in_=xt[:, P:2 * P], identity=ident[:])
                xT = wp.tile([P, IN_P], f32, tag="xT")
                nc.vector.tensor_copy(xT[:, 0:P], tp0[:])
                nc.vector.tensor_copy(xT[:, P:2 * P], tp1[:])
                hm = psA.tile([P, H], f32, tag="M")
                nc.tensor.matmul(out=hm[:], lhsT=xT[:, 0:P], rhs=w_emb_p0[:],
                                 start=True, stop=False)
                nc.tensor.matmul(out=hm[:], lhsT=xT[:, P:2 * P], rhs=w_emb_p1[:],
                                 start=False, stop=True)
                dst = h_p[:, t * H:(t + 1) * H]
                if emb_p_b:
                    tb = wp.tile([P, H], f32, tag="ebt")
                    nc.vector.tensor_add(tb[:], hm[:], OPT["emb_p_b"][:])
                    nc.scalar.activation(out=dst, in_=tb[:], func=AF.Relu)
                else:
                    nc.scalar.activation(out=dst, in_=hm[:], func=AF.Relu)
            for t in range(AT):
                xt = wp.tile([P, IN_A], f32, tag="xt")
                nc.sync.dma_start(xt[:], d_in["x_a"][t * P:(t + 1) * P, :])
                xT = transpose_to_sbuf(xt[:, 0:P], "xTa")
                hm = psA.tile([P, H], f32, tag="M")
                nc.tensor.matmul(out=hm[:], lhsT=xT[:], rhs=w_emb_a[:],
                                 start=True, stop=True)
                dst = h_a[:, t * H:(t + 1) * H]
                if emb_a_b:
                    tb = wp.tile([P, H], f32, tag="ebt")
                    nc.vector.tensor_add(tb[:], hm[:], OPT["emb_a_b"][:])
                    nc.scalar.activation(out=dst, in_=tb[:], func=AF.Relu)
                else:
                    nc.scalar.activation(out=dst, in_=hm[:], func=AF.Relu)

            if DBG:
                for t in range(PT):
                    nc.sync.dma_start(d_emb_p[t * P:(t + 1) * P, :],
                                      h_p[:, t * H:(t + 1) * H])

            # ---- layer body helpers ----
            def f_pass(l, n_tiles, n_rows, h_sb, w_gat, w_u, asr, f_gat_dram,
                       ald_dram, w_sage=None, f_sage_dram=None):
                for t in range(n_tiles):
                    rows = min(P, n_rows - t * P)
                    hT = transpose_to_sbuf(h_sb[:, t * H:(t + 1) * H], "hT")
                    fg = psB.tile([P, 260], f32, tag="F")
                    nc.tensor.matmul(out=fg[:, 0:H], lhsT=hT[:], rhs=w_gat[:],
                                     start=True, stop=True)
                    nc.tensor.matmul(out=fg[:, H:H + 4], lhsT=hT[:], rhs=w_u[:],
                                     start=True, stop=True)
                    als_m = wp.tile([P, H], f32, tag="alsm")
                    nc.vector.tensor_tensor(out=h4(als_m[:]), in0=h4(fg[:, 0:H]),
                                            in1=h4(asr[:]), op=OP.mult)
                    stage = wp.tile([P, 132], f32, tag="fstage")
                    nc.scalar.activation(out=stage[:, 0:H], in_=fg[:, 0:H],
                                         func=AF.Identity)
                    nc.vector.reduce_sum(
                        out=stage[:, H:H + 4].unsqueeze(2),
                        in_=h4(als_m[:]), axis=mybir.AxisListType.X)
                    nc.sync.dma_start(f_gat_dram[t * P:t * P + rows, :],
                                      stage[:rows, :])
                    alds = wp.tile([P, 4], f32, tag="alds")
                    nc.vector.tensor_copy(alds[:], fg[:, H:H + 4])
                    nc.sync.dma_start(ald_dram[t * P:t * P + rows, :],
                                      alds[:rows, :])
                    if w_sage is not None:
                        nc.tensor.matmul(out=fg[:, 132:260], lhsT=hT[:],
                                         rhs=w_sage[:], start=True, stop=True)
                        st2 = wp.tile([P, H], f32, tag="fstage2")
                        nc.scalar.activation(out=st2[:], in_=fg[:, 132:260],
                                             func=AF.Identity)
                        nc.sync.dma_start(f_sage_dram[t * P:t * P + rows, :],
                                          st2[:rows, :])

            def gat_chunks(l, t, K, off, srcm, relm, locm, f_full, ald_dram, acc):
                for k in range(int(K[t])):
                    q = int(off[t]) + k
                    G = gp.tile([P, 132], f32, tag="G")
                    nc.gpsimd.indirect_dma_start(
                        out=G[:], out_offset=None, in_=f_full[:],
                        in_offset=bass.IndirectOffsetOnAxis(
                            ap=srcm[:, q:q + 1], axis=0))
                    ald_e = gp.tile([P, 4], f32, tag="alde")
                    nc.gpsimd.indirect_dma_start(
                        out=ald_e[:], out_offset=None, in_=ald_dram[:],
                        in_offset=bass.IndirectOffsetOnAxis(
                            ap=locm[:, q:q + 1], axis=0))
                    S = gp.tile([P, P], f32, tag="S")
                    nc.vector.tensor_scalar(out=S[:], in0=iota[:],
                                            scalar1=relm[:, q:q + 1], scalar2=None,
                                            op0=OP.is_equal)
                    e4 = gp.tile([P, 4], f32, tag="e4")
                    nc.vector.tensor_add(e4[:], G[:, H:H + 4], ald_e[:])
                    e4b = gp.tile([P, 4], f32, tag="e4b")
                    nc.vector.tensor_scalar(out=e4b[:], in0=e4[:], scalar1=0.2,
                                            scalar2=None, op0=OP.mult)
                    nc.vector.tensor_tensor(out=e4b[:], in0=e4[:], in1=e4b[:],
                                            op=OP.max)
                    msgx = gp.tile([P, H + 4], f32, tag="msg")
                    nc.scalar.activation(out=msgx[:, H:H + 4], in_=e4b[:],
                                         func=AF.Exp)
                    nc.vector.tensor_tensor(
                        out=h4(msgx[:, 0:H]), in0=h4(G[:, 0:H]),
                        in1=msgx[:, H:H + 4].unsqueeze(2).broadcast_to(
                            [P, HEADS, CH]),
                        op=OP.mult)
                    nc.tensor.matmul(out=acc[:, 0:H + 4], lhsT=S[:], rhs=msgx[:],
                                     start=(k == 0), stop=(k == int(K[t]) - 1))

            def layer_norm_relu_resid(comb, h_sb, t, lng, lnb):
                mus = wp.tile([P, 1], f32, tag="mus")
                nc.vector.reduce_sum(out=mus[:].unsqueeze(2),
                                     in_=comb[:].unsqueeze(1),
                                     axis=mybir.AxisListType.X)
                mu = wp.tile([P, 1], f32, tag="mu")
                nc.vector.tensor_scalar(out=mu[:], in0=mus[:], scalar1=1.0 / H,
                                        scalar2=None, op0=OP.mult)
                nc.vector.tensor_scalar(out=comb[:], in0=comb[:],
                                        scalar1=mu[:, 0:1], scalar2=None,
                                        op0=OP.subtract)
                sqj = wp.tile([P, H], f32, tag="sqj")
                vs = wp.tile([P, 1], f32, tag="vs")
                nc.scalar.activation(out=sqj[:], in_=comb[:], func=AF.Square,
                                     accum_out=vs[:])
                std = wp.tile([P, 1], f32, tag="std")
                nc.scalar.activation(out=std[:], in_=vs[:], func=AF.Sqrt,
                                     scale=1.0 / H, bias=eps_t[:, 0:1])
                rstd = wp.tile([P, 1], f32, tag="rstd")
                nc.vector.reciprocal(rstd[:], std[:])
                nc.vector.tensor_scalar(out=comb[:], in0=comb[:],
                                        scalar1=rstd[:, 0:1], scalar2=None,
                                        op0=OP.mult)
                if lng is not None:
                    nc.vector.tensor_tensor(out=comb[:], in0=comb[:], in1=lng[:],
                                            op=OP.mult)
                if lnb is not None:
                    nc.vector.tensor_add(comb[:], comb[:], lnb[:])
                r = wp.tile([P, H], f32, tag="lnr")
                nc.scalar.activation(out=r[:], in_=comb[:], func=AF.Relu)
                dst = h_sb[:, t * H:(t + 1) * H]
                nc.vector.tensor_add(dst, r[:], dst)

            # ---- layers ----
            for l in range(L):
                if DBG and l == 1:
                    for t in range(PT):
                        nc.sync.dma_start(d_hp1[t * P:(t + 1) * P, :],
                                          h_p[:, t * H:(t + 1) * H])
                    for t in range(AT):
                        nc.sync.dma_start(d_ha1[t * P:(t + 1) * P, :],
                                          h_a[:, t * H:(t + 1) * H])
                f_pass(l, PT, PSH, h_p, WS[f"wsa{l}"], WS[f"uw{l}"], WS[f"asa{l}"],
                       fpg_in[l], aldp[l], WS[f"wl{l}"], fps_in[l])
                f_pass(l, AT, ASH, h_a, WS[f"wsw{l}"], WS[f"ua{l}"], WS[f"asw{l}"],
                       fa_in[l], alda[l])
                nc.gpsimd.collective_compute(
                    "AllGather", OP.bypass, replica_groups=RG,
                    ins=[fa_in[l][:]], outs=[fa_full[l][:]])
                nc.gpsimd.collective_compute(
                    "AllGather", OP.bypass, replica_groups=RG,
                    ins=[fps_in[l][:]], outs=[fps_full[l][:]])
                nc.gpsimd.collective_compute(
                    "AllGather", OP.bypass, replica_groups=RG,
                    ins=[fpg_in[l][:]], outs=[fpg_full[l][:]])

                if DBG and l == 0:
                    bwork = wp.tile([P, 132], f32, tag="dbgcopy", name="bwork")
                    nc.sync.dma_start(bwork[:], fa_full[l][0:P, :])
                    nc.sync.dma_start(d_fa[0:P, :], bwork[:])
                    for blk in range(0, NA, 4096):
                        n = min(4096, NA - blk)
                        nc.sync.dma_start(d_fa[blk:blk + n, :],
                                          fa_full[l][blk:blk + n, :])
                    for blk in range(0, NP_, 4096):
                        n = min(4096, NP_ - blk)
                        nc.sync.dma_start(d_fpg[blk:blk + n, :],
                                          fpg_full[l][blk:blk + n, :])
                        nc.sync.dma_start(d_fps[blk:blk + n, :],
                                          fps_full[l][blk:blk + n, :])
                    nc.sync.dma_start(d_aldp[:, :], aldp[l][:, :])

                # papers: writes-GAT + cites-SAGE + combine
                for t in range(PT):
                    comb = wp.tile([P, H], f32, tag="comb")
                    if Kw[t] > 0:
                        acc = psB.tile([P, 132], f32, tag="ACC")
                        gat_chunks(l, t, Kw, offw, META["src_w"], META["rel_w"],
                                   META["loc_w"], fa_full[l], aldp[l], acc)
                        s4 = wp.tile([P, 4], f32, tag="s4")
                        nc.vector.tensor_scalar(out=s4[:], in0=acc[:, H:H + 4],
                                                scalar1=1e-16, scalar2=None,
                                                op0=OP.add)
                        rec = wp.tile([P, 4], f32, tag="rec")
                        nc.vector.reciprocal(rec[:], s4[:])
                        nc.vector.tensor_tensor(
                            out=h4(comb[:]), in0=h4(acc[:, 0:H]),
                            in1=rec[:].unsqueeze(2).broadcast_to([P, HEADS, CH]),
                            op=OP.mult)
                    else:
                        nc.gpsimd.memset(comb[:], 0.0)
                    if Kc[t] > 0:
                        agg = psB.tile([P, H], f32, tag="ACC", name="agg")
                        for k in range(int(Kc[t])):
                            q = int(offc[t]) + k
                            Gs = gp.tile([P, H], f32, tag="Gs")
                            nc.gpsimd.indirect_dma_start(
                                out=Gs[:], out_offset=None, in_=fps_full[l][:],
                                in_offset=bass.IndirectOffsetOnAxis(
                                    ap=META["src_c"][:, q:q + 1], axis=0))
                            Ss = gp.tile([P, P], f32, tag="S")
                            nc.vector.tensor_scalar(out=Ss[:], in0=iota[:],
                                                    scalar1=META["rel_c"][:, q:q + 1],
                                                    scalar2=None, op0=OP.is_equal)
                            nc.tensor.matmul(out=agg[:], lhsT=Ss[:], rhs=Gs[:],
                                             start=(k == 0),
                                             stop=(k == int(Kc[t]) - 1))
                        mn = wp.tile([P, H], f32, tag="mn")
                        nc.vector.tensor_scalar(out=mn[:], in0=agg[:],
                                                scalar1=META["rcp_c"][:, t:t + 1],
                                                scalar2=None, op0=OP.mult)
                        nc.vector.tensor_add(comb[:], comb[:], mn[:])
                    # + h_dst @ Wr
                    hT2 = transpose_to_sbuf(h_p[:, t * H:(t + 1) * H], "hT2")
                    wrp = psA.tile([P, H], f32, tag="M")
                    nc.tensor.matmul(out=wrp[:], lhsT=hT2[:], rhs=WS[f"wr{l}"][:],
                                     start=True, stop=True)
                    nc.vector.tensor_add(comb[:], comb[:], wrp[:])
                    if use_bias_p[l]:
                        nc.vector.tensor_add(comb[:], comb[:], OPT[f"bias_p{l}"][:])
                    layer_norm_relu_resid(
                        comb, h_p, t,
                        OPT[f"lng_p{l}"] if use_ln_g[l][0] else None,
                        OPT[f"lnb_p{l}"] if use_ln_b[l][0] else None)

                # authors: auth-GAT + combine
                for t in range(AT):
                    comb = wp.tile([P, H], f32, tag="comb")
                    if Ka[t] > 0:
                        acc = psB.tile([P, 132], f32, tag="ACC")
                        gat_chunks(l, t, Ka, offa, META["src_a"], META["rel_a"],
                                   META["loc_a"], fpg_full[l], alda[l], acc)
                        s4 = wp.tile([P, 4], f32, tag="s4")
                        nc.vector.tensor_scalar(out=s4[:], in0=acc[:, H:H + 4],
                                                scalar1=1e-16, scalar2=None,
                                                op0=OP.add)
                        rec = wp.tile([P, 4], f32, tag="rec")
                        nc.vector.reciprocal(rec[:], s4[:])
                        nc.vector.tensor_tensor(
                            out=h4(comb[:]), in0=h4(acc[:, 0:H]),
                            in1=rec[:].unsqueeze(2).broadcast_to([P, HEADS, CH]),
                            op=OP.mult)
                    else:
                        nc.gpsimd.memset(comb[:], 0.0)
                    if use_bias_a[l]:
                        nc.vector.tensor_add(comb[:], comb[:], OPT[f"bias_a{l}"][:])
                    layer_norm_relu_resid(
                        comb, h_a, t,
                        OPT[f"lng_a{l}"] if use_ln_g[l][1] else None,
                        OPT[f"lnb_a{l}"] if use_ln_b[l][1] else None)

            # ---- output projections ----
            for t in range(PT):
                rows = min(P, PSH - t * P)
                hT = transpose_to_sbuf(h_p[:, t * H:(t + 1) * H], "hTo")
                om = psA.tile([P, H], f32, tag="M")
                nc.tensor.matmul(out=om[:], lhsT=hT[:], rhs=w_out_p[:],
                                 start=True, stop=True)
                ost = wp.tile([P, H], f32, tag="ost")
                if out_p_b:
                    nc.vector.tensor_add(ost[:], om[:], OPT["out_p_b"][:])
                else:
                    nc.scalar.activation(out=ost[:], in_=om[:], func=AF.Identity)
                nc.sync.dma_start(o_p[t * P:t * P + rows, :], ost[:rows, :])
            for t in range(AT):
                rows = min(P, ASH - t * P)
                hT = transpose_to_sbuf(h_a[:, t * H:(t + 1) * H], "hTo")
                om = psA.tile([P, H], f32, tag="M")
                nc.tensor.matmul(out=om[:], lhsT=hT[:], rhs=w_out_a[:],
                                 start=True, stop=True)
                ost = wp.tile([P, H], f32, tag="ost")
                if out_a_b:
                    nc.vector.tensor_add(ost[:], om[:], OPT["out_a_b"][:])
                else:
                    nc.scalar.activation(out=ost[:], in_=om[:], func=AF.Identity)
                nc.sync.dma_start(o_a[t * P:t * P + rows, :], ost[:rows, :])

    nc.compile()
    res = run_bass_kernel_spmd(nc, in_maps, list(range(NCORES)))
    out_author = np.concatenate([res.results[r]["o_a"] for r in range(NCORES)], 0)
    out_paper = np.concatenate([res.results[r]["o_p"] for r in range(NCORES)], 0)
    return out_author, out_paper


def kernel(**inputs):
    return _build_and_run(inputs)


# revision 9
# speedup vs baseline: 1.5206x; 1.5206x over previous
"""Trainium2 Bass kernel for the heterogeneous GNN (GAT + SAGE, 2 layers).

Strategy: destination-node sharding across 8 cores (papers 12500/core,
authors 6250/core). Each layer:
  1. Per-core dense pass over the LOCAL node shard computes projected
     gather-tables:  F_a  = [h_a @ Wsrc_w | al_s_w]        (132 f32/row)
                     F_pg = [h_p @ Wsrc_auth | al_s_auth]  (132)
                     F_ps = [h_p @ Wl_cites]               (128)
     plus local attention dst-terms al_d (h @ (Wdst·adst)).
  2. AllGather the three tables (all cores get full copies).
  3. Edge phase: edges are pre-sorted by destination on the host and cut
     into 128-edge chunks per 128-destination tile. Per chunk: one
     indirect-DMA row gather from the F table, a selection matrix
     S[e,j] = (dst_rel[e] == j) built on the vector engine, softmax
     numerators exp(leakyrelu(al_s+al_d)) (max-shift dropped - softmax is
     shift invariant), messages scaled per head, then ONE matmul
     S.T @ msg accumulating into PSUM (plus a 4-wide matmul accumulating
     softmax denominators into the same PSUM tile's tail columns).
  4. Softmax division per destination after aggregation, SAGE mean via
     host-precomputed reciprocal counts, + h_dst @ Wr, LayerNorm, ReLU,
     residual - all local to the shard (h kept SBUF-resident).
Weight tensors are replicated; biases/ln params that are zero/one in the
given inputs elide their ops at program-build time.
"""
import sys

if "/opt/trn_rl_repo" not in sys.path:
    sys.path.insert(0, "/opt/trn_rl_repo")

import numpy as np

NCORES = 8
NA, NP_ = 50000, 100000
H, HEADS, CH = 128, 4, 32
IN_A, IN_P = 128, 256
LN_EPS = 1e-5
L = 2
P = 128
PSH, ASH = NP_ // NCORES, NA // NCORES          # 12500, 6250
PT, AT = (PSH + P - 1) // P, (ASH + P - 1) // P  # 98, 49


def _prep_edges(src, dst, shard, n_tiles, need_loc):
    """Sort edges by dst, shard by dst owner, cut into per-tile 128-edge
    chunks padded to a chunk count that is uniform ACROSS CORES (the SPMD
    program is shared). Returns (K[t] per tile, per-core dicts of
    [128, Q] arrays: src ids, dst_rel f32, dst local idx)."""
    src = np.asarray(src).astype(np.int64)
    dst = np.asarray(dst).astype(np.int64)
    per_core = []
    cnts = np.zeros((NCORES, n_tiles), np.int64)
    for r in range(NCORES):
        lo, hi = r * shard, (r + 1) * shard
        sel = (dst >= lo) & (dst < hi)
        s, d = src[sel], dst[sel] - lo
        o = np.argsort(d, kind="stable")
        s, d = s[o], d[o]
        t = d >> 7
        cnts[r] = np.bincount(t, minlength=n_tiles)
        per_core.append((s, d))
    K = ((cnts + P - 1) // P).max(axis=0)          # chunks per tile
    off = np.concatenate([[0], np.cumsum(K)]).astype(np.int64)
    Q = int(off[-1])
    out = []
    for r in range(NCORES):
        s, d = per_core[r]
        src_a = np.zeros((P, Q), np.int32)
        rel_a = np.full((P, Q), -1.0, np.float32)
        loc_a = np.zeros((P, Q), np.int32)
        bounds = np.concatenate([[0], np.cumsum(cnts[r])])
        for t in range(n_tiles):
            b0, b1 = bounds[t], bounds[t + 1]
            n = b1 - b0
            if n == 0:
                continue
            j = np.arange(n)
            col = off[t] + (j >> 7)
            row = j & 127
            src_a[row, col] = s[b0:b1]
            rel_a[row, col] = (d[b0:b1] - t * P).astype(np.float32)
            loc_a[row, col] = d[b0:b1]
        ent = {"src": src_a, "rel": rel_a}
        if need_loc:
            ent["loc"] = loc_a
        out.append(ent)
    return K.astype(np.int64), off, Q, out


def _build_and_run(inp):
    import concourse.bass as bass
    import concourse.mybir as mybir
    import concourse.tile as tile
    from concourse import bacc
    from concourse.masks import make_identity
    from concourse.bass_utils import run_bass_kernel_spmd

    f32, i32 = mybir.dt.float32, mybir.dt.int32

    g = lambda k: np.asarray(inp[k], np.float32)
    gi = lambda k: np.asarray(inp[k], np.int32)

    # ---------- host prep: edges ----------
    Kw, offw, Qw, ew = _prep_edges(gi("writes_src"), gi("writes_dst"), PSH, PT, True)
    Kc, offc, Qc, ec = _prep_edges(gi("cites_src"), gi("cites_dst"), PSH, PT, False)
    Ka, offa, Qa, ea = _prep_edges(gi("auth_src"), gi("auth_dst"), ASH, AT, True)

    # SAGE reciprocal counts per destination paper, tile-column layout
    cnt = np.bincount(gi("cites_dst"), minlength=NP_).astype(np.float32)
    rcp = 1.0 / np.maximum(cnt, 1.0)
    rcp_T = np.ones((P, PT * NCORES), np.float32)
    for r in range(NCORES):
        blk = rcp[r * PSH:(r + 1) * PSH]
        blk = np.pad(blk, (0, PT * P - PSH), constant_values=1.0)
        rcp_T[:, r * PT:(r + 1) * PT] = blk.reshape(PT, P).T

    # ---------- host prep: weights ----------
    iota_np = np.tile(np.arange(P, dtype=np.float32), (P, 1))
    wdict = {"iota": iota_np,
             "w_emb_a": g("emb_author_W"), "w_emb_p": g("emb_paper_W"),
             "w_out_a": g("out_author_W"), "w_out_p": g("out_paper_W")}
    for l in range(L):
        wdict[f"wsw{l}"] = g("gat_writes_Wsrc")[l]
        wdict[f"wsa{l}"] = g("gat_auth_Wsrc")[l]
        wdict[f"wl{l}"] = g("sage_cites_Wl")[l]
        wdict[f"wr{l}"] = g("sage_cites_Wr")[l]
        for nm, wk, ak in (("uw", "gat_writes_Wdst", "gat_writes_adst"),
                           ("ua", "gat_auth_Wdst", "gat_auth_adst")):
            W, a = g(wk)[l], g(ak)[l]
            wdict[f"{nm}{l}"] = (W.reshape(H, HEADS, CH) * a[None]).sum(-1)  # [H,4]
        wdict[f"asw{l}"] = np.tile(g("gat_writes_asrc")[l].reshape(1, H), (P, 1))
        wdict[f"asa{l}"] = np.tile(g("gat_auth_asrc")[l].reshape(1, H), (P, 1))

    # optional bias / ln tiles (elided when trivial)
    def rep(v):
        return np.tile(np.asarray(v, np.float32).reshape(1, H), (P, 1))
    nz = lambda v: not np.all(np.asarray(v) == 0.0)
    none1 = lambda v: not np.all(np.asarray(v) == 1.0)
    emb_a_b, emb_p_b = nz(inp["emb_author_b"]), nz(inp["emb_paper_b"])
    out_a_b, out_p_b = nz(inp["out_author_b"]), nz(inp["out_paper_b"])
    bias_p = [g("gat_writes_b")[l] + g("sage_cites_bl")[l] + g("sage_cites_br")[l]
              for l in range(L)]
    bias_a = [g("gat_auth_b")[l] for l in range(L)]
    use_bias_p = [nz(b) for b in bias_p]
    use_bias_a = [nz(b) for b in bias_a]
    use_ln_g = [[none1(g("ln_paper_g")[l]), none1(g("ln_author_g")[l])] for l in range(L)]
    use_ln_b = [[nz(g("ln_paper_b")[l]), nz(g("ln_author_b")[l])] for l in range(L)]
    for l in range(L):
        if emb_a_b: wdict["emb_a_b"] = rep(inp["emb_author_b"])
        if emb_p_b: wdict["emb_p_b"] = rep(inp["emb_paper_b"])
        if out_a_b: wdict["out_a_b"] = rep(inp["out_author_b"])
        if out_p_b: wdict["out_p_b"] = rep(inp["out_paper_b"])
        if use_bias_p[l]: wdict[f"bias_p{l}"] = rep(bias_p[l])
        if use_bias_a[l]: wdict[f"bias_a{l}"] = rep(bias_a[l])
        if use_ln_g[l][0]: wdict[f"lng_p{l}"] = rep(g("ln_paper_g")[l])
        if use_ln_g[l][1]: wdict[f"lng_a{l}"] = rep(g("ln_author_g")[l])
        if use_ln_b[l][0]: wdict[f"lnb_p{l}"] = rep(g("ln_paper_b")[l])
        if use_ln_b[l][1]: wdict[f"lnb_a{l}"] = rep(g("ln_author_b")[l])

    # per-core inputs
    xa = g("x_author"); xp = g("x_paper")
    in_maps = []
    for r in range(NCORES):
        m = dict(wdict)
        m["x_a"] = np.pad(xa[r * ASH:(r + 1) * ASH], ((0, AT * P - ASH), (0, 0)))
        m["x_p"] = np.pad(xp[r * PSH:(r + 1) * PSH], ((0, PT * P - PSH), (0, 0)))
        m["src_w"], m["rel_w"], m["loc_w"] = ew[r]["src"], ew[r]["rel"], ew[r]["loc"]
        m["src_c"], m["rel_c"] = ec[r]["src"], ec[r]["rel"]
        m["src_a"], m["rel_a"], m["loc_a"] = ea[r]["src"], ea[r]["rel"], ea[r]["loc"]
        m["rcp_c"] = np.ascontiguousarray(rcp_T[:, r * PT:(r + 1) * PT])
        in_maps.append(m)

    # ---------- program ----------
    nc = bacc.Bacc("TRN2", target_bir_lowering=False, debug=False,
                   num_devices=NCORES)
    ein = lambda n, s, dt=f32: nc.dram_tensor(n, s, dt, kind="ExternalInput").ap()
    eout = lambda n, s: nc.dram_tensor(n, s, f32, kind="ExternalOutput").ap()

    d_in = {k: ein(k, list(v.shape), i32 if v.dtype == np.int32 else f32)
            for k, v in in_maps[0].items()}
    o_a = eout("o_a", [ASH, H])
    o_p = eout("o_p", [PSH, H])
    import os as _os
    DBG = int(_os.environ.get("KDBG", "0"))
    if DBG:
        d_emb_p = eout("d_emb_p", [PT * P, H])
        d_fa = eout("d_fa", [NA, 132])
        d_fpg = eout("d_fpg", [NP_, 132])
        d_fps = eout("d_fps", [NP_, H])
        d_hp1 = eout("d_hp1", [PT * P, H])
        d_ha1 = eout("d_ha1", [AT * P, H])

    fa_in = [nc.dram_tensor(f"fa_in{l}", [ASH, 132], f32).ap() for l in range(L)]
    fpg_in = [nc.dram_tensor(f"fpg_in{l}", [PSH, 132], f32).ap() for l in range(L)]
    fps_in = [nc.dram_tensor(f"fps_in{l}", [PSH, H], f32).ap() for l in range(L)]
    fa_full = [nc.dram_tensor(f"fa_full{l}", [NA, 132], f32,
                              addr_space="Shared").ap() for l in range(L)]
    fpg_full = [nc.dram_tensor(f"fpg_full{l}", [NP_, 132], f32,
                               addr_space="Shared").ap() for l in range(L)]
    fps_full = [nc.dram_tensor(f"fps_full{l}", [NP_, H], f32,
                               addr_space="Shared").ap() for l in range(L)]

    RG = [list(range(NCORES))]
    AF = mybir.ActivationFunctionType
    OP = mybir.AluOpType
    h4 = lambda ap: ap.rearrange("p (h c) -> p h c", h=HEADS)

    with tile.TileContext(nc) as tc:
        with tc.tile_pool(name="const", bufs=1) as cp, \
             tc.tile_pool(name="meta", bufs=1) as mp, \
             tc.tile_pool(name="work", bufs=3) as wp, \
             tc.tile_pool(name="gat", bufs=6) as gp, \
             tc.tile_pool(name="psA", bufs=3, space="PSUM") as psA, \
             tc.tile_pool(name="psB", bufs=2, space="PSUM") as psB:

            # ---- resident constants ----
            def cload(name, shape=None, dt=f32):
                t = cp.tile(shape or list(in_maps[0][name].shape), dt, tag=name)
                nc.sync.dma_start(t[:], d_in[name][:])
                return t
            ident = cp.tile([P, P], f32, tag="ident")
            make_identity(nc, ident[:])
            eps_t = cp.tile([P, 1], f32, tag="epsc")
            nc.gpsimd.memset(eps_t[:], LN_EPS)
            iota = cload("iota")
            w_emb_a = cload("w_emb_a")
            w_emb_p0 = cp.tile([P, H], f32, tag="wep0")
            w_emb_p1 = cp.tile([P, H], f32, tag="wep1")
            nc.sync.dma_start(w_emb_p0[:], d_in["w_emb_p"][0:P, :])
            nc.sync.dma_start(w_emb_p1[:], d_in["w_emb_p"][P:2 * P, :])
            w_out_a, w_out_p = cload("w_out_a"), cload("w_out_p")
            WS = {k: cload(k) for k in
                  [f"{n}{l}" for l in range(L)
                   for n in ("wsw", "wsa", "wl", "wr", "uw", "ua", "asw", "asa")]}
            OPT = {k: cload(k) for k in wdict if k.startswith(("bias_", "lng_",
                                                               "lnb_", "emb_", "out_"))
                   if k in d_in and k not in ("out_a", "out_p")}
            # edge metadata + counts
            META = {k: mp.tile(list(in_maps[0][k].shape),
                               i32 if in_maps[0][k].dtype == np.int32 else f32,
                               tag=k, name=k)
                    for k in ("src_w", "rel_w", "loc_w", "src_c", "rel_c",
                              "src_a", "rel_a", "loc_a", "rcp_c")}
            for k, t in META.items():
                nc.sync.dma_start(t[:], d_in[k][:])
            # resident node states
            h_p = cp.tile([P, PT * H], f32, tag="h_p")
            h_a = cp.tile([P, AT * H], f32, tag="h_a")
            ald_p = cp.tile([P, PT * 4], f32, tag="ald_p")
            ald_a = cp.tile([P, AT * 4], f32, tag="ald_a")

            def transpose_to_sbuf(src_ap, tag):
                tp = psA.tile([P, P], f32, tag="T")
                nc.tensor.transpose(out=tp[:], in_=src_ap, identity=ident[:])
                sb = wp.tile([P, P], f32, tag=tag)
                nc.vector.tensor_copy(sb[:], tp[:])
                return sb

            # ---- embeddings ----
            for t in range(PT):
                xt = wp.tile([P, IN_P], f32, tag="xt")
                nc.sync.dma_start(xt[:], d_in["x_p"][t * P:(t + 1) * P, :])
                tp0 = psA.tile([P, P], f32, tag="T")
                nc.tensor.transpose(out=tp0[:], in_=xt[:, 0:P], identity=ident[:])
                tp1 = psA.tile([P, P], f32, tag="T")
                nc.tensor.transpose(out=tp1[:], in_=xt[:, P:2 * P], identity=ident[:])
                xT = wp.tile([P, IN_P], f32, tag="xT")
                nc.vector.tensor_copy(xT[:, 0:P], tp0[:])
                nc.vector.tensor_copy(xT[:, P:2 * P], tp1[:])
                hm = psA.tile([P, H], f32, tag="T")
                nc.tensor.matmul(out=hm[:], lhsT=xT[:, 0:P], rhs=w_emb_p0[:],
                                 start=True, stop=False)
                nc.tensor.matmul(out=hm[:], lhsT=xT[:, P:2 * P], rhs=w_emb_p1[:],
                                 start=False, stop=True)
                dst = h_p[:, t * H:(t + 1) * H]
                if emb_p_b:
                    tb = wp.tile([P, H], f32, tag="ebt")
                    nc.vector.tensor_add(tb[:], hm[:], OPT["emb_p_b"][:])
                    nc.scalar.activation(out=dst, in_=tb[:], func=AF.Relu)
                else:
                    nc.scalar.activation(out=dst, in_=hm[:], func=AF.Relu)
            for t in range(AT):
                xt = wp.tile([P, IN_A], f32, tag="xt")
                nc.sync.dma_start(xt[:], d_in["x_a"][t * P:(t + 1) * P, :])
                xT = transpose_to_sbuf(xt[:, 0:P], "xTa")
                hm = psA.tile([P, H], f32, tag="T")
                nc.tensor.matmul(out=hm[:], lhsT=xT[:], rhs=w_emb_a[:],
                                 start=True, stop=True)
                dst = h_a[:, t * H:(t + 1) * H]
                if emb_a_b:
                    tb = wp.tile([P, H], f32, tag="ebt")
                    nc.vector.tensor_add(tb[:], hm[:], OPT["emb_a_b"][:])
                    nc.scalar.activation(out=dst, in_=tb[:], func=AF.Relu)
                else:
                    nc.scalar.activation(out=dst, in_=hm[:], func=AF.Relu)

            if DBG:
                for t in range(PT):
                    nc.sync.dma_start(d_emb_p[t * P:(t + 1) * P, :],
                                      h_p[:, t * H:(t + 1) * H])

            # ---- layer body helpers ----
            def f_pass(l, n_tiles, n_rows, h_sb, w_gat, w_u, asr, f_gat_dram,
                       ald_sb, w_sage=None, f_sage_dram=None):
                for t in range(n_tiles):
                    rows = min(P, n_rows - t * P)
                    hT = transpose_to_sbuf(h_sb[:, t * H:(t + 1) * H], "hT")
                    fg = psB.tile([P, 260], f32, tag="F")
                    nc.tensor.matmul(out=fg[:, 0:H], lhsT=hT[:], rhs=w_gat[:],
                                     start=True, stop=True)
                    nc.tensor.matmul(out=fg[:, H:H + 4], lhsT=hT[:], rhs=w_u[:],
                                     start=True, stop=True)
                    als_m = wp.tile([P, H], f32, tag="alsm")
                    nc.vector.tensor_tensor(out=h4(als_m[:]), in0=h4(fg[:, 0:H]),
                                            in1=h4(asr[:]), op=OP.mult)
                    stage = wp.tile([P, 132], f32, tag="fstage")
                    nc.scalar.activation(out=stage[:, 0:H], in_=fg[:, 0:H],
                                         func=AF.Identity)
                    nc.vector.reduce_sum(
                        out=stage[:, H:H + 4].unsqueeze(2),
                        in_=h4(als_m[:]), axis=mybir.AxisListType.X)
                    nc.sync.dma_start(f_gat_dram[t * P:t * P + rows, :],
                                      stage[:rows, :])
                    nc.vector.tensor_copy(ald_sb[:, t * 4:(t + 1) * 4],
                                          fg[:, H:H + 4])
                    if w_sage is not None:
                        nc.tensor.matmul(out=fg[:, 132:260], lhsT=hT[:],
                                         rhs=w_sage[:], start=True, stop=True)
                        st2 = wp.tile([P, H], f32, tag="fstage2")
                        nc.scalar.activation(out=st2[:], in_=fg[:, 132:260],
                                             func=AF.Identity)
                        nc.sync.dma_start(f_sage_dram[t * P:t * P + rows, :],
                                          st2[:rows, :])

            def gat_chunks(l, t, K, off, srcm, relm, ald_col, f_full, acc):
                for k in range(int(K[t])):
                    q = int(off[t]) + k
                    G = gp.tile([P, 132], f32, tag="G")
                    nc.gpsimd.indirect_dma_start(
                        out=G[:], out_offset=None, in_=f_full[:],
                        in_offset=bass.IndirectOffsetOnAxis(
                            ap=srcm[:, q:q + 1], axis=0))
                    S = gp.tile([P, P], f32, tag="S")
                    nc.vector.tensor_scalar(out=S[:], in0=iota[:],
                                            scalar1=relm[:, q:q + 1], scalar2=None,
                                            op0=OP.is_equal)
                    stp = psA.tile([P, P], f32, tag="T", name="stp")
                    nc.tensor.transpose(out=stp[:], in_=S[:], identity=ident[:])
                    st = gp.tile([P, P], f32, tag="ST")
                    nc.scalar.activation(out=st[:], in_=stp[:], func=AF.Identity)
                    aldps = psB.tile([P, 4], f32, tag="F", name="aldps")
                    nc.tensor.matmul(out=aldps[:], lhsT=st[:], rhs=ald_col,
                                     start=True, stop=True)
                    e4 = gp.tile([P, 4], f32, tag="e4")
                    nc.vector.tensor_add(e4[:], G[:, H:H + 4], aldps[:])
                    e4b = gp.tile([P, 4], f32, tag="e4b")
                    nc.vector.tensor_scalar(out=e4b[:], in0=e4[:], scalar1=0.2,
                                            scalar2=None, op0=OP.mult)
                    nc.vector.tensor_tensor(out=e4b[:], in0=e4[:], in1=e4b[:],
                                            op=OP.max)
                    msgx = gp.tile([P, H + 4], f32, tag="msg")
                    nc.scalar.activation(out=msgx[:, H:H + 4], in_=e4b[:],
                                         func=AF.Exp)
                    nc.vector.tensor_tensor(
                        out=h4(msgx[:, 0:H]), in0=h4(G[:, 0:H]),
                        in1=msgx[:, H:H + 4].unsqueeze(2).broadcast_to(
                            [P, HEADS, CH]),
                        op=OP.mult)
                    nc.tensor.matmul(out=acc[:, 0:H + 4], lhsT=S[:], rhs=msgx[:],
                                     start=(k == 0), stop=(k == int(K[t]) - 1))

            def layer_norm_relu_resid(comb, h_sb, t, lng, lnb):
                mus = wp.tile([P, 1], f32, tag="mus")
                nc.vector.reduce_sum(out=mus[:].unsqueeze(2),
                                     in_=comb[:].unsqueeze(1),
                                     axis=mybir.AxisListType.X)
                mu = wp.tile([P, 1], f32, tag="mu")
                nc.vector.tensor_scalar(out=mu[:], in0=mus[:], scalar1=1.0 / H,
                                        scalar2=None, op0=OP.mult)
                nc.vector.tensor_scalar(out=comb[:], in0=comb[:],
                                        scalar1=mu[:, 0:1], scalar2=None,
                                        op0=OP.subtract)
                sqj = wp.tile([P, H], f32, tag="sqj")
                vs = wp.tile([P, 1], f32, tag="vs")
                nc.scalar.activation(out=sqj[:], in_=comb[:], func=AF.Square,
                                     accum_out=vs[:])
                std = wp.tile([P, 1], f32, tag="std")
                nc.scalar.activation(out=std[:], in_=vs[:], func=AF.Sqrt,
                                     scale=1.0 / H, bias=eps_t[:, 0:1])
                rstd = wp.tile([P, 1], f32, tag="rstd")
                nc.vector.reciprocal(rstd[:], std[:])
                nc.vector.tensor_scalar(out=comb[:], in0=comb[:],
                                        scalar1=rstd[:, 0:1], scalar2=None,
                                        op0=OP.mult)
                if lng is not None:
                    nc.vector.tensor_tensor(out=comb[:], in0=comb[:], in1=lng[:],
                                            op=OP.mult)
                if lnb is not None:
                    nc.vector.tensor_add(comb[:], comb[:], lnb[:])
                r = wp.tile([P, H], f32, tag="lnr")
                nc.scalar.activation(out=r[:], in_=comb[:], func=AF.Relu)
                dst = h_sb[:, t * H:(t + 1) * H]
                nc.vector.tensor_add(dst, r[:], dst)

            # ---- layers ----
            for l in range(L):
                if DBG and l == 1:
                    for t in range(PT):
                        nc.sync.dma_start(d_hp1[t * P:(t + 1) * P, :],
                                          h_p[:, t * H:(t + 1) * H])
                    for t in range(AT):
                        nc.sync.dma_start(d_ha1[t * P:(t + 1) * P, :],
                                          h_a[:, t * H:(t + 1) * H])
                f_pass(l, PT, PSH, h_p, WS[f"wsa{l}"], WS[f"uw{l}"], WS[f"asa{l}"],
                       fpg_in[l], ald_p, WS[f"wl{l}"], fps_in[l])
                f_pass(l, AT, ASH, h_a, WS[f"wsw{l}"], WS[f"ua{l}"], WS[f"asw{l}"],
                       fa_in[l], ald_a)
                nc.gpsimd.collective_compute(
                    "AllGather", OP.bypass, replica_groups=RG,
                    ins=[fa_in[l][:]], outs=[fa_full[l][:]])
                nc.gpsimd.collective_compute(
                    "AllGather", OP.bypass, replica_groups=RG,
                    ins=[fps_in[l][:]], outs=[fps_full[l][:]])
                nc.gpsimd.collective_compute(
                    "AllGather", OP.bypass, replica_groups=RG,
                    ins=[fpg_in[l][:]], outs=[fpg_full[l][:]])

                if DBG and l == 0:
                    bwork = wp.tile([P, 132], f32, tag="dbgcopy", name="bwork")
                    nc.sync.dma_start(bwork[:], fa_full[l][0:P, :])
                    nc.sync.dma_start(d_fa[0:P, :], bwork[:])
                    for blk in range(0, NA, 4096):
                        n = min(4096, NA - blk)
                        nc.sync.dma_start(d_fa[blk:blk + n, :],
                                          fa_full[l][blk:blk + n, :])
                    for blk in range(0, NP_, 4096):
                        n = min(4096, NP_ - blk)
                        nc.sync.dma_start(d_fpg[blk:blk + n, :],
                                          fpg_full[l][blk:blk + n, :])
                        nc.sync.dma_start(d_fps[blk:blk + n, :],
                                          fps_full[l][blk:blk + n, :])

                # papers: writes-GAT + cites-SAGE + combine
                for t in range(PT):
                    comb = wp.tile([P, H], f32, tag="comb")
                    if Kw[t] > 0:
                        acc = psB.tile([P, 132], f32, tag="ACC", bufs=3)
                        gat_chunks(l, t, Kw, offw, META["src_w"], META["rel_w"],
                                   ald_p[:, t * 4:(t + 1) * 4], fa_full[l], acc)
                        s4 = wp.tile([P, 4], f32, tag="s4")
                        nc.vector.tensor_scalar(out=s4[:], in0=acc[:, H:H + 4],
                                                scalar1=1e-16, scalar2=None,
                                                op0=OP.add)
                        rec = wp.tile([P, 4], f32, tag="rec")
                        nc.vector.reciprocal(rec[:], s4[:])
                        nc.vector.tensor_tensor(
                            out=h4(comb[:]), in0=h4(acc[:, 0:H]),
                            in1=rec[:].unsqueeze(2).broadcast_to([P, HEADS, CH]),
                            op=OP.mult)
                    else:
                        nc.gpsimd.memset(comb[:], 0.0)
                    if Kc[t] > 0:
                        agg = psB.tile([P, H], f32, tag="ACC", name="agg", bufs=3)
                        for k in range(int(Kc[t])):
                            q = int(offc[t]) + k
                            Gs = gp.tile([P, H], f32, tag="Gs")
                            nc.gpsimd.indirect_dma_start(
                                out=Gs[:], out_offset=None, in_=fps_full[l][:],
                                in_offset=bass.IndirectOffsetOnAxis(
                                    ap=META["src_c"][:, q:q + 1], axis=0))
                            Ss = gp.tile([P, P], f32, tag="S")
                            nc.vector.tensor_scalar(out=Ss[:], in0=iota[:],
                                                    scalar1=META["rel_c"][:, q:q + 1],
                                                    scalar2=None, op0=OP.is_equal)
                            nc.tensor.matmul(out=agg[:], lhsT=Ss[:], rhs=Gs[:],
                                             start=(k == 0),
                                             stop=(k == int(Kc[t]) - 1))
                        mn = wp.tile([P, H], f32, tag="mn")
                        nc.vector.tensor_scalar(out=mn[:], in0=agg[:],
                                                scalar1=META["rcp_c"][:, t:t + 1],
                                                scalar2=None, op0=OP.mult)
                        nc.vector.tensor_add(comb[:], comb[:], mn[:])
                    # + h_dst @ Wr
                    hT2 = transpose_to_sbuf(h_p[:, t * H:(t + 1) * H], "hT2")
                    wrp = psA.tile([P, H], f32, tag="T")
                    nc.tensor.matmul(out=wrp[:], lhsT=hT2[:], rhs=WS[f"wr{l}"][:],
                                     start=True, stop=True)
                    nc.vector.tensor_add(comb[:], comb[:], wrp[:])
                    if use_bias_p[l]:
                        nc.vector.tensor_add(comb[:], comb[:], OPT[f"bias_p{l}"][:])
                    layer_norm_relu_resid(
                        comb, h_p, t,
                        OPT[f"lng_p{l}"] if use_ln_g[l][0] else None,
                        OPT[f"lnb_p{l}"] if use_ln_b[l][0] else None)

                # authors: auth-GAT + combine
                for t in range(AT):
                    comb = wp.tile([P, H], f32, tag="comb")
                    if Ka[t] > 0:
                        acc = psB.tile([P, 132], f32, tag="ACC", bufs=3)
                        gat_chunks(l, t, Ka, offa, META["src_a"], META["rel_a"],
                                   ald_a[:, t * 4:(t + 1) * 4], fpg_full[l], acc)
                        s4 = wp.tile([P, 4], f32, tag="s4")
                        nc.vector.tensor_scalar(out=s4[:], in0=acc[:, H:H + 4],
                                                scalar1=1e-16, scalar2=None,
                                                op0=OP.add)
                        rec = wp.tile([P, 4], f32, tag="rec")
                        nc.vector.reciprocal(rec[:], s4[:])
                        nc.vector.tensor_tensor(
                            out=h4(comb[:]), in0=h4(acc[:, 0:H]),
                            in1=rec[:].unsqueeze(2).broadcast_to([P, HEADS, CH]),
                            op=OP.mult)
                    else:
                        nc.gpsimd.memset(comb[:], 0.0)
                    if use_bias_a[l]:
                        nc.vector.tensor_add(comb[:], comb[:], OPT[f"bias_a{l}"][:])
                    layer_norm_relu_resid(
                        comb, h_a, t,
                        OPT[f"lng_a{l}"] if use_ln_g[l][1] else None,
                        OPT[f"lnb_a{l}"] if use_ln_b[l][1] else None)

            # ---- output projections ----
            for t in range(PT):
                rows = min(P, PSH - t * P)
                hT = transpose_to_sbuf(h_p[:, t * H:(t + 1) * H], "hTo")
                om = psA.tile([P, H], f32, tag="T")
                nc.tensor.matmul(out=om[:], lhsT=hT[:], rhs=w_out_p[:],
                                 start=True, stop=True)
                ost = wp.tile([P, H], f32, tag="ost")
                if out_p_b:
                    nc.vector.tensor_add(ost[:], om[:], OPT["out_p_b"][:])
                else:
                    nc.scalar.activation(out=ost[:], in_=om[:], func=AF.Identity)
                nc.sync.dma_start(o_p[t * P:t * P + rows, :], ost[:rows, :])
            for t in range(AT):
                rows = min(P, ASH - t * P)
                hT = transpose_to_sbuf(h_a[:, t * H:(t + 1) * H], "hTo")
                om = psA.tile([P, H], f32, tag="T")
                nc.tensor.matmul(out=om[:], lhsT=hT[:], rhs=w_out_a[:],
                                 start=True, stop=True)
                ost = wp.tile([P, H], f32, tag="ost")
                if out_a_b:
                    nc.vector.tensor_add(ost[:], om[:], OPT["out_a_b"][:])
                else:
                    nc.scalar.activation(out=ost[:], in_=om[:], func=AF.Identity)
                nc.sync.dma_start(o_a[t * P:t * P + rows, :], ost[:rows, :])

    nc.compile()
    res = run_bass_kernel_spmd(nc, in_maps, list(range(NCORES)))
    out_author = np.concatenate([res.results[r]["o_a"] for r in range(NCORES)], 0)
    out_paper = np.concatenate([res.results[r]["o_p"] for r in range(NCORES)], 0)
    return out_author, out_paper


def kernel(**inputs):
    return _build_and_run(inputs)


# revision 10
# speedup vs baseline: 1.5274x; 1.0045x over previous
"""Trainium2 Bass kernel for the heterogeneous GNN (GAT + SAGE, 2 layers).

Strategy: destination-node sharding across 8 cores (papers 12500/core,
authors 6250/core). Each layer:
  1. Per-core dense pass over the LOCAL node shard computes projected
     gather-tables:  F_a  = [h_a @ Wsrc_w | al_s_w]        (132 f32/row)
                     F_pg = [h_p @ Wsrc_auth | al_s_auth]  (132)
                     F_ps = [h_p @ Wl_cites]               (128)
     plus local attention dst-terms al_d (h @ (Wdst·adst)).
  2. AllGather the three tables (all cores get full copies).
  3. Edge phase: edges are pre-sorted by destination on the host and cut
     into 128-edge chunks per 128-destination tile. Per chunk: one
     indirect-DMA row gather from the F table, a selection matrix
     S[e,j] = (dst_rel[e] == j) built on the vector engine, softmax
     numerators exp(leakyrelu(al_s+al_d)) (max-shift dropped - softmax is
     shift invariant), messages scaled per head, then ONE matmul
     S.T @ msg accumulating into PSUM (plus a 4-wide matmul accumulating
     softmax denominators into the same PSUM tile's tail columns).
  4. Softmax division per destination after aggregation, SAGE mean via
     host-precomputed reciprocal counts, + h_dst @ Wr, LayerNorm, ReLU,
     residual - all local to the shard (h kept SBUF-resident).
Weight tensors are replicated; biases/ln params that are zero/one in the
given inputs elide their ops at program-build time.
"""
import sys

if "/opt/trn_rl_repo" not in sys.path:
    sys.path.insert(0, "/opt/trn_rl_repo")

import numpy as np

NCORES = 8
NA, NP_ = 50000, 100000
H, HEADS, CH = 128, 4, 32
IN_A, IN_P = 128, 256
LN_EPS = 1e-5
L = 2
P = 128
PSH, ASH = NP_ // NCORES, NA // NCORES          # 12500, 6250
PT, AT = (PSH + P - 1) // P, (ASH + P - 1) // P  # 98, 49


def _prep_edges(src, dst, shard, n_tiles, need_loc):
    """Sort edges by dst, shard by dst owner, cut into per-tile 128-edge
    chunks padded to a chunk count that is uniform ACROSS CORES (the SPMD
    program is shared). Returns (K[t] per tile, per-core dicts of
    [128, Q] arrays: src ids, dst_rel f32, dst local idx)."""
    src = np.asarray(src).astype(np.int64)
    dst = np.asarray(dst).astype(np.int64)
    per_core = []
    cnts = np.zeros((NCORES, n_tiles), np.int64)
    for r in range(NCORES):
        lo, hi = r * shard, (r + 1) * shard
        sel = (dst >= lo) & (dst < hi)
        s, d = src[sel], dst[sel] - lo
        o = np.argsort(d, kind="stable")
        s, d = s[o], d[o]
        t = d >> 7
        cnts[r] = np.bincount(t, minlength=n_tiles)
        per_core.append((s, d))
    K = ((cnts + P - 1) // P).max(axis=0)          # chunks per tile
    off = np.concatenate([[0], np.cumsum(K)]).astype(np.int64)
    Q = int(off[-1])
    out = []
    for r in range(NCORES):
        s, d = per_core[r]
        src_a = np.zeros((P, Q), np.int32)
        rel_a = np.full((P, Q), -1.0, np.float32)
        loc_a = np.zeros((P, Q), np.int32)
        bounds = np.concatenate([[0], np.cumsum(cnts[r])])
        for t in range(n_tiles):
            b0, b1 = bounds[t], bounds[t + 1]
            n = b1 - b0
            if n == 0:
                continue
            j = np.arange(n)
            col = off[t] + (j >> 7)
            row = j & 127
            src_a[row, col] = s[b0:b1]
            rel_a[row, col] = (d[b0:b1] - t * P).astype(np.float32)
            loc_a[row, col] = d[b0:b1]
        ent = {"src": src_a, "rel": rel_a}
        if need_loc:
            ent["loc"] = loc_a
        out.append(ent)
    return K.astype(np.int64), off, Q, out


def _build_and_run(inp):
    import concourse.bass as bass
    import concourse.mybir as mybir
    import concourse.tile as tile
    from concourse import bacc
    from concourse.masks import make_identity
    from concourse.bass_utils import run_bass_kernel_spmd

    f32, i32 = mybir.dt.float32, mybir.dt.int32

    g = lambda k: np.asarray(inp[k], np.float32)
    gi = lambda k: np.asarray(inp[k], np.int32)

    # ---------- host prep: edges ----------
    Kw, offw, Qw, ew = _prep_edges(gi("writes_src"), gi("writes_dst"), PSH, PT, True)
    Kc, offc, Qc, ec = _prep_edges(gi("cites_src"), gi("cites_dst"), PSH, PT, False)
    Ka, offa, Qa, ea = _prep_edges(gi("auth_src"), gi("auth_dst"), ASH, AT, True)

    # SAGE reciprocal counts per destination paper, tile-column layout
    cnt = np.bincount(gi("cites_dst"), minlength=NP_).astype(np.float32)
    rcp = 1.0 / np.maximum(cnt, 1.0)
    rcp_T = np.ones((P, PT * NCORES), np.float32)
    for r in range(NCORES):
        blk = rcp[r * PSH:(r + 1) * PSH]
        blk = np.pad(blk, (0, PT * P - PSH), constant_values=1.0)
        rcp_T[:, r * PT:(r + 1) * PT] = blk.reshape(PT, P).T

    # ---------- host prep: weights ----------
    iota_np = np.tile(np.arange(P, dtype=np.float32), (P, 1))
    wdict = {"iota": iota_np,
             "w_emb_a": g("emb_author_W"), "w_emb_p": g("emb_paper_W"),
             "w_out_a": g("out_author_W"), "w_out_p": g("out_paper_W")}
    for l in range(L):
        wdict[f"wsw{l}"] = g("gat_writes_Wsrc")[l]
        wdict[f"wsa{l}"] = g("gat_auth_Wsrc")[l]
        wdict[f"wl{l}"] = g("sage_cites_Wl")[l]
        wdict[f"wr{l}"] = g("sage_cites_Wr")[l]
        for nm, wk, ak in (("uw", "gat_writes_Wdst", "gat_writes_adst"),
                           ("ua", "gat_auth_Wdst", "gat_auth_adst")):
            W, a = g(wk)[l], g(ak)[l]
            wdict[f"{nm}{l}"] = (W.reshape(H, HEADS, CH) * a[None]).sum(-1)  # [H,4]
        wdict[f"asw{l}"] = np.tile(g("gat_writes_asrc")[l].reshape(1, H), (P, 1))
        wdict[f"asa{l}"] = np.tile(g("gat_auth_asrc")[l].reshape(1, H), (P, 1))

    # optional bias / ln tiles (elided when trivial)
    def rep(v):
        return np.tile(np.asarray(v, np.float32).reshape(1, H), (P, 1))
    nz = lambda v: not np.all(np.asarray(v) == 0.0)
    none1 = lambda v: not np.all(np.asarray(v) == 1.0)
    emb_a_b, emb_p_b = nz(inp["emb_author_b"]), nz(inp["emb_paper_b"])
    out_a_b, out_p_b = nz(inp["out_author_b"]), nz(inp["out_paper_b"])
    bias_p = [g("gat_writes_b")[l] + g("sage_cites_bl")[l] + g("sage_cites_br")[l]
              for l in range(L)]
    bias_a = [g("gat_auth_b")[l] for l in range(L)]
    use_bias_p = [nz(b) for b in bias_p]
    use_bias_a = [nz(b) for b in bias_a]
    use_ln_g = [[none1(g("ln_paper_g")[l]), none1(g("ln_author_g")[l])] for l in range(L)]
    use_ln_b = [[nz(g("ln_paper_b")[l]), nz(g("ln_author_b")[l])] for l in range(L)]
    for l in range(L):
        if emb_a_b: wdict["emb_a_b"] = rep(inp["emb_author_b"])
        if emb_p_b: wdict["emb_p_b"] = rep(inp["emb_paper_b"])
        if out_a_b: wdict["out_a_b"] = rep(inp["out_author_b"])
        if out_p_b: wdict["out_p_b"] = rep(inp["out_paper_b"])
        if use_bias_p[l]: wdict[f"bias_p{l}"] = rep(bias_p[l])
        if use_bias_a[l]: wdict[f"bias_a{l}"] = rep(bias_a[l])
        if use_ln_g[l][0]: wdict[f"lng_p{l}"] = rep(g("ln_paper_g")[l])
        if use_ln_g[l][1]: wdict[f"lng_a{l}"] = rep(g("ln_author_g")[l])
        if use_ln_b[l][0]: wdict[f"lnb_p{l}"] = rep(g("ln_paper_b")[l])
        if use_ln_b[l][1]: wdict[f"lnb_a{l}"] = rep(g("ln_author_b")[l])

    # per-core inputs
    xa = g("x_author"); xp = g("x_paper")
    in_maps = []
    for r in range(NCORES):
        m = dict(wdict)
        m["x_a"] = np.pad(xa[r * ASH:(r + 1) * ASH], ((0, AT * P - ASH), (0, 0)))
        m["x_p"] = np.pad(xp[r * PSH:(r + 1) * PSH], ((0, PT * P - PSH), (0, 0)))
        m["src_w"], m["rel_w"], m["loc_w"] = ew[r]["src"], ew[r]["rel"], ew[r]["loc"]
        m["src_c"], m["rel_c"] = ec[r]["src"], ec[r]["rel"]
        m["src_a"], m["rel_a"], m["loc_a"] = ea[r]["src"], ea[r]["rel"], ea[r]["loc"]
        m["rcp_c"] = np.ascontiguousarray(rcp_T[:, r * PT:(r + 1) * PT])
        in_maps.append(m)

    # ---------- program ----------
    nc = bacc.Bacc("TRN2", target_bir_lowering=False, debug=False,
                   num_devices=NCORES)
    ein = lambda n, s, dt=f32: nc.dram_tensor(n, s, dt, kind="ExternalInput").ap()
    eout = lambda n, s: nc.dram_tensor(n, s, f32, kind="ExternalOutput").ap()

    d_in = {k: ein(k, list(v.shape), i32 if v.dtype == np.int32 else f32)
            for k, v in in_maps[0].items()}
    o_a = eout("o_a", [ASH, H])
    o_p = eout("o_p", [PSH, H])
    import os as _os
    DBG = int(_os.environ.get("KDBG", "0"))
    if DBG:
        d_emb_p = eout("d_emb_p", [PT * P, H])
        d_fa = eout("d_fa", [NA, 132])
        d_fpg = eout("d_fpg", [NP_, 132])
        d_fps = eout("d_fps", [NP_, H])
        d_hp1 = eout("d_hp1", [PT * P, H])
        d_ha1 = eout("d_ha1", [AT * P, H])

    fa_in = [nc.dram_tensor(f"fa_in{l}", [ASH, 132], f32).ap() for l in range(L)]
    fpg_in = [nc.dram_tensor(f"fpg_in{l}", [PSH, 132], f32).ap() for l in range(L)]
    fps_in = [nc.dram_tensor(f"fps_in{l}", [PSH, H], f32).ap() for l in range(L)]
    fa_full = [nc.dram_tensor(f"fa_full{l}", [NA, 132], f32,
                              addr_space="Shared").ap() for l in range(L)]
    fpg_full = [nc.dram_tensor(f"fpg_full{l}", [NP_, 132], f32,
                               addr_space="Shared").ap() for l in range(L)]
    fps_full = [nc.dram_tensor(f"fps_full{l}", [NP_, H], f32,
                               addr_space="Shared").ap() for l in range(L)]

    RG = [list(range(NCORES))]
    AF = mybir.ActivationFunctionType
    OP = mybir.AluOpType
    h4 = lambda ap: ap.rearrange("p (h c) -> p h c", h=HEADS)

    with tile.TileContext(nc) as tc:
        with tc.tile_pool(name="const", bufs=1) as cp, \
             tc.tile_pool(name="meta", bufs=1) as mp, \
             tc.tile_pool(name="work", bufs=3) as wp, \
             tc.tile_pool(name="gat", bufs=10) as gp, \
             tc.tile_pool(name="psA", bufs=3, space="PSUM") as psA, \
             tc.tile_pool(name="psB", bufs=2, space="PSUM") as psB:

            # ---- resident constants ----
            def cload(name, shape=None, dt=f32):
                t = cp.tile(shape or list(in_maps[0][name].shape), dt, tag=name)
                nc.sync.dma_start(t[:], d_in[name][:])
                return t
            ident = cp.tile([P, P], f32, tag="ident")
            make_identity(nc, ident[:])
            eps_t = cp.tile([P, 1], f32, tag="epsc")
            nc.gpsimd.memset(eps_t[:], LN_EPS)
            iota = cload("iota")
            w_emb_a = cload("w_emb_a")
            w_emb_p0 = cp.tile([P, H], f32, tag="wep0")
            w_emb_p1 = cp.tile([P, H], f32, tag="wep1")
            nc.sync.dma_start(w_emb_p0[:], d_in["w_emb_p"][0:P, :])
            nc.sync.dma_start(w_emb_p1[:], d_in["w_emb_p"][P:2 * P, :])
            w_out_a, w_out_p = cload("w_out_a"), cload("w_out_p")
            WS = {k: cload(k) for k in
                  [f"{n}{l}" for l in range(L)
                   for n in ("wsw", "wsa", "wl", "wr", "uw", "ua", "asw", "asa")]}
            OPT = {k: cload(k) for k in wdict if k.startswith(("bias_", "lng_",
                                                               "lnb_", "emb_", "out_"))
                   if k in d_in and k not in ("out_a", "out_p")}
            # edge metadata + counts
            META = {k: mp.tile(list(in_maps[0][k].shape),
                               i32 if in_maps[0][k].dtype == np.int32 else f32,
                               tag=k, name=k)
                    for k in ("src_w", "rel_w", "loc_w", "src_c", "rel_c",
                              "src_a", "rel_a", "loc_a", "rcp_c")}
            for k, t in META.items():
                nc.sync.dma_start(t[:], d_in[k][:])
            # resident node states
            h_p = cp.tile([P, PT * H], f32, tag="h_p")
            h_a = cp.tile([P, AT * H], f32, tag="h_a")
            ald_p = cp.tile([P, PT * 4], f32, tag="ald_p")
            ald_a = cp.tile([P, AT * 4], f32, tag="ald_a")

            def transpose_to_sbuf(src_ap, tag):
                tp = psA.tile([P, P], f32, tag="T")
                nc.tensor.transpose(out=tp[:], in_=src_ap, identity=ident[:])
                sb = wp.tile([P, P], f32, tag=tag)
                nc.vector.tensor_copy(sb[:], tp[:])
                return sb

            # ---- embeddings ----
            for t in range(PT):
                xt = wp.tile([P, IN_P], f32, tag="xt")
                nc.sync.dma_start(xt[:], d_in["x_p"][t * P:(t + 1) * P, :])
                tp0 = psA.tile([P, P], f32, tag="T")
                nc.tensor.transpose(out=tp0[:], in_=xt[:, 0:P], identity=ident[:])
                tp1 = psA.tile([P, P], f32, tag="T")
                nc.tensor.transpose(out=tp1[:], in_=xt[:, P:2 * P], identity=ident[:])
                xT = wp.tile([P, IN_P], f32, tag="xT")
                nc.vector.tensor_copy(xT[:, 0:P], tp0[:])
                nc.vector.tensor_copy(xT[:, P:2 * P], tp1[:])
                hm = psA.tile([P, H], f32, tag="T")
                nc.tensor.matmul(out=hm[:], lhsT=xT[:, 0:P], rhs=w_emb_p0[:],
                                 start=True, stop=False)
                nc.tensor.matmul(out=hm[:], lhsT=xT[:, P:2 * P], rhs=w_emb_p1[:],
                                 start=False, stop=True)
                dst = h_p[:, t * H:(t + 1) * H]
                if emb_p_b:
                    tb = wp.tile([P, H], f32, tag="ebt")
                    nc.vector.tensor_add(tb[:], hm[:], OPT["emb_p_b"][:])
                    nc.scalar.activation(out=dst, in_=tb[:], func=AF.Relu)
                else:
                    nc.scalar.activation(out=dst, in_=hm[:], func=AF.Relu)
            for t in range(AT):
                xt = wp.tile([P, IN_A], f32, tag="xt")
                nc.sync.dma_start(xt[:], d_in["x_a"][t * P:(t + 1) * P, :])
                xT = transpose_to_sbuf(xt[:, 0:P], "xTa")
                hm = psA.tile([P, H], f32, tag="T")
                nc.tensor.matmul(out=hm[:], lhsT=xT[:], rhs=w_emb_a[:],
                                 start=True, stop=True)
                dst = h_a[:, t * H:(t + 1) * H]
                if emb_a_b:
                    tb = wp.tile([P, H], f32, tag="ebt")
                    nc.vector.tensor_add(tb[:], hm[:], OPT["emb_a_b"][:])
                    nc.scalar.activation(out=dst, in_=tb[:], func=AF.Relu)
                else:
                    nc.scalar.activation(out=dst, in_=hm[:], func=AF.Relu)

            if DBG:
                for t in range(PT):
                    nc.sync.dma_start(d_emb_p[t * P:(t + 1) * P, :],
                                      h_p[:, t * H:(t + 1) * H])

            # ---- layer body helpers ----
            def f_pass(l, n_tiles, n_rows, h_sb, w_gat, w_u, asr, f_gat_dram,
                       ald_sb, w_sage=None, f_sage_dram=None):
                for t in range(n_tiles):
                    rows = min(P, n_rows - t * P)
                    hT = transpose_to_sbuf(h_sb[:, t * H:(t + 1) * H], "hT")
                    fg = psB.tile([P, 260], f32, tag="F")
                    nc.tensor.matmul(out=fg[:, 0:H], lhsT=hT[:], rhs=w_gat[:],
                                     start=True, stop=True)
                    nc.tensor.matmul(out=fg[:, H:H + 4], lhsT=hT[:], rhs=w_u[:],
                                     start=True, stop=True)
                    als_m = wp.tile([P, H], f32, tag="alsm")
                    nc.vector.tensor_tensor(out=h4(als_m[:]), in0=h4(fg[:, 0:H]),
                                            in1=h4(asr[:]), op=OP.mult)
                    stage = wp.tile([P, 132], f32, tag="fstage")
                    nc.scalar.activation(out=stage[:, 0:H], in_=fg[:, 0:H],
                                         func=AF.Identity)
                    nc.vector.reduce_sum(
                        out=stage[:, H:H + 4].unsqueeze(2),
                        in_=h4(als_m[:]), axis=mybir.AxisListType.X)
                    nc.sync.dma_start(f_gat_dram[t * P:t * P + rows, :],
                                      stage[:rows, :])
                    nc.vector.tensor_copy(ald_sb[:, t * 4:(t + 1) * 4],
                                          fg[:, H:H + 4])
                    if w_sage is not None:
                        nc.tensor.matmul(out=fg[:, 132:260], lhsT=hT[:],
                                         rhs=w_sage[:], start=True, stop=True)
                        st2 = wp.tile([P, H], f32, tag="fstage2")
                        nc.scalar.activation(out=st2[:], in_=fg[:, 132:260],
                                             func=AF.Identity)
                        nc.sync.dma_start(f_sage_dram[t * P:t * P + rows, :],
                                          st2[:rows, :])

            def gat_chunks(l, t, K, off, srcm, relm, ald_col, f_full, acc):
                for k in range(int(K[t])):
                    q = int(off[t]) + k
                    G = gp.tile([P, 132], f32, tag="G")
                    nc.gpsimd.indirect_dma_start(
                        out=G[:], out_offset=None, in_=f_full[:],
                        in_offset=bass.IndirectOffsetOnAxis(
                            ap=srcm[:, q:q + 1], axis=0))
                    S = gp.tile([P, P], f32, tag="S")
                    nc.vector.tensor_scalar(out=S[:], in0=iota[:],
                                            scalar1=relm[:, q:q + 1], scalar2=None,
                                            op0=OP.is_equal)
                    stp = psA.tile([P, P], f32, tag="T", name="stp")
                    nc.tensor.transpose(out=stp[:], in_=S[:], identity=ident[:])
                    st = gp.tile([P, P], f32, tag="ST")
                    nc.vector.tensor_copy(st[:], stp[:])
                    aldps = psB.tile([P, 4], f32, tag="F", name="aldps")
                    nc.tensor.matmul(out=aldps[:], lhsT=st[:], rhs=ald_col,
                                     start=True, stop=True)
                    e4 = gp.tile([P, 4], f32, tag="e4")
                    nc.vector.tensor_add(e4[:], G[:, H:H + 4], aldps[:])
                    e4b = gp.tile([P, 4], f32, tag="e4b")
                    nc.vector.tensor_scalar(out=e4b[:], in0=e4[:], scalar1=0.2,
                                            scalar2=None, op0=OP.mult)
                    nc.vector.tensor_tensor(out=e4b[:], in0=e4[:], in1=e4b[:],
                                            op=OP.max)
                    msgx = gp.tile([P, H + 4], f32, tag="msg")
                    nc.scalar.activation(out=msgx[:, H:H + 4], in_=e4b[:],
                                         func=AF.Exp)
                    nc.vector.tensor_tensor(
                        out=h4(msgx[:, 0:H]), in0=h4(G[:, 0:H]),
                        in1=msgx[:, H:H + 4].unsqueeze(2).broadcast_to(
                            [P, HEADS, CH]),
                        op=OP.mult)
                    nc.tensor.matmul(out=acc[:, 0:H + 4], lhsT=S[:], rhs=msgx[:],
                                     start=(k == 0), stop=(k == int(K[t]) - 1))

            def layer_norm_relu_resid(comb, h_sb, t, lng, lnb):
                mus = wp.tile([P, 1], f32, tag="mus")
                nc.vector.reduce_sum(out=mus[:].unsqueeze(2),
                                     in_=comb[:].unsqueeze(1),
                                     axis=mybir.AxisListType.X)
                mu = wp.tile([P, 1], f32, tag="mu")
                nc.vector.tensor_scalar(out=mu[:], in0=mus[:], scalar1=1.0 / H,
                                        scalar2=None, op0=OP.mult)
                nc.vector.tensor_scalar(out=comb[:], in0=comb[:],
                                        scalar1=mu[:, 0:1], scalar2=None,
                                        op0=OP.subtract)
                sqj = wp.tile([P, H], f32, tag="sqj")
                vs = wp.tile([P, 1], f32, tag="vs")
                nc.scalar.activation(out=sqj[:], in_=comb[:], func=AF.Square,
                                     accum_out=vs[:])
                std = wp.tile([P, 1], f32, tag="std")
                nc.scalar.activation(out=std[:], in_=vs[:], func=AF.Sqrt,
                                     scale=1.0 / H, bias=eps_t[:, 0:1])
                rstd = wp.tile([P, 1], f32, tag="rstd")
                nc.vector.reciprocal(rstd[:], std[:])
                nc.vector.tensor_scalar(out=comb[:], in0=comb[:],
                                        scalar1=rstd[:, 0:1], scalar2=None,
                                        op0=OP.mult)
                if lng is not None:
                    nc.vector.tensor_tensor(out=comb[:], in0=comb[:], in1=lng[:],
                                            op=OP.mult)
                if lnb is not None:
                    nc.vector.tensor_add(comb[:], comb[:], lnb[:])
                r = wp.tile([P, H], f32, tag="lnr")
                nc.vector.tensor_scalar(out=r[:], in0=comb[:], scalar1=0.0,
                                        scalar2=None, op0=OP.max)
                dst = h_sb[:, t * H:(t + 1) * H]
                nc.vector.tensor_add(dst, r[:], dst)

            # ---- layers ----
            for l in range(L):
                if DBG and l == 1:
                    for t in range(PT):
                        nc.sync.dma_start(d_hp1[t * P:(t + 1) * P, :],
                                          h_p[:, t * H:(t + 1) * H])
                    for t in range(AT):
                        nc.sync.dma_start(d_ha1[t * P:(t + 1) * P, :],
                                          h_a[:, t * H:(t + 1) * H])
                f_pass(l, PT, PSH, h_p, WS[f"wsa{l}"], WS[f"uw{l}"], WS[f"asa{l}"],
                       fpg_in[l], ald_p, WS[f"wl{l}"], fps_in[l])
                f_pass(l, AT, ASH, h_a, WS[f"wsw{l}"], WS[f"ua{l}"], WS[f"asw{l}"],
                       fa_in[l], ald_a)
                nc.gpsimd.collective_compute(
                    "AllGather", OP.bypass, replica_groups=RG,
                    ins=[fa_in[l][:]], outs=[fa_full[l][:]])
                nc.gpsimd.collective_compute(
                    "AllGather", OP.bypass, replica_groups=RG,
                    ins=[fps_in[l][:]], outs=[fps_full[l][:]])
                nc.gpsimd.collective_compute(
                    "AllGather", OP.bypass, replica_groups=RG,
                    ins=[fpg_in[l][:]], outs=[fpg_full[l][:]])

                if DBG and l == 0:
                    bwork = wp.tile([P, 132], f32, tag="dbgcopy", name="bwork")
                    nc.sync.dma_start(bwork[:], fa_full[l][0:P, :])
                    nc.sync.dma_start(d_fa[0:P, :], bwork[:])
                    for blk in range(0, NA, 4096):
                        n = min(4096, NA - blk)
                        nc.sync.dma_start(d_fa[blk:blk + n, :],
                                          fa_full[l][blk:blk + n, :])
                    for blk in range(0, NP_, 4096):
                        n = min(4096, NP_ - blk)
                        nc.sync.dma_start(d_fpg[blk:blk + n, :],
                                          fpg_full[l][blk:blk + n, :])
                        nc.sync.dma_start(d_fps[blk:blk + n, :],
                                          fps_full[l][blk:blk + n, :])

                # papers: writes-GAT + cites-SAGE + combine
                for t in range(PT):
                    comb = wp.tile([P, H], f32, tag="comb")
                    if Kw[t] > 0:
                        acc = psB.tile([P, 132], f32, tag="ACC", bufs=3)
                        gat_chunks(l, t, Kw, offw, META["src_w"], META["rel_w"],
                                   ald_p[:, t * 4:(t + 1) * 4], fa_full[l], acc)
                        s4 = wp.tile([P, 4], f32, tag="s4")
                        nc.vector.tensor_scalar(out=s4[:], in0=acc[:, H:H + 4],
                                                scalar1=1e-16, scalar2=None,
                                                op0=OP.add)
                        rec = wp.tile([P, 4], f32, tag="rec")
                        nc.vector.reciprocal(rec[:], s4[:])
                        nc.vector.tensor_tensor(
                            out=h4(comb[:]), in0=h4(acc[:, 0:H]),
                            in1=rec[:].unsqueeze(2).broadcast_to([P, HEADS, CH]),
                            op=OP.mult)
                    else:
                        nc.gpsimd.memset(comb[:], 0.0)
                    if Kc[t] > 0:
                        agg = psB.tile([P, H], f32, tag="ACC", name="agg", bufs=3)
                        for k in range(int(Kc[t])):
                            q = int(offc[t]) + k
                            Gs = gp.tile([P, H], f32, tag="Gs")
                            nc.gpsimd.indirect_dma_start(
                                out=Gs[:], out_offset=None, in_=fps_full[l][:],
                                in_offset=bass.IndirectOffsetOnAxis(
                                    ap=META["src_c"][:, q:q + 1], axis=0))
                            Ss = gp.tile([P, P], f32, tag="S")
                            nc.vector.tensor_scalar(out=Ss[:], in0=iota[:],
                                                    scalar1=META["rel_c"][:, q:q + 1],
                                                    scalar2=None, op0=OP.is_equal)
                            nc.tensor.matmul(out=agg[:], lhsT=Ss[:], rhs=Gs[:],
                                             start=(k == 0),
                                             stop=(k == int(Kc[t]) - 1))
                        mn = wp.tile([P, H], f32, tag="mn")
                        nc.vector.tensor_scalar(out=mn[:], in0=agg[:],
                                                scalar1=META["rcp_c"][:, t:t + 1],
                                                scalar2=None, op0=OP.mult)
                        nc.vector.tensor_add(comb[:], comb[:], mn[:])
                    # + h_dst @ Wr
                    hT2 = transpose_to_sbuf(h_p[:, t * H:(t + 1) * H], "hT2")
                    wrp = psA.tile([P, H], f32, tag="T")
                    nc.tensor.matmul(out=wrp[:], lhsT=hT2[:], rhs=WS[f"wr{l}"][:],
                                     start=True, stop=True)
                    nc.vector.tensor_add(comb[:], comb[:], wrp[:])
                    if use_bias_p[l]:
                        nc.vector.tensor_add(comb[:], comb[:], OPT[f"bias_p{l}"][:])
                    layer_norm_relu_resid(
                        comb, h_p, t,
                        OPT[f"lng_p{l}"] if use_ln_g[l][0] else None,
                        OPT[f"lnb_p{l}"] if use_ln_b[l][0] else None)

                # authors: auth-GAT + combine
                for t in range(AT):
                    comb = wp.tile([P, H], f32, tag="comb")
                    if Ka[t] > 0:
                        acc = psB.tile([P, 132], f32, tag="ACC", bufs=3)
                        gat_chunks(l, t, Ka, offa, META["src_a"], META["rel_a"],
                                   ald_a[:, t * 4:(t + 1) * 4], fpg_full[l], acc)
                        s4 = wp.tile([P, 4], f32, tag="s4")
                        nc.vector.tensor_scalar(out=s4[:], in0=acc[:, H:H + 4],
                                                scalar1=1e-16, scalar2=None,
                                                op0=OP.add)
                        rec = wp.tile([P, 4], f32, tag="rec")
                        nc.vector.reciprocal(rec[:], s4[:])
                        nc.vector.tensor_tensor(
                            out=h4(comb[:]), in0=h4(acc[:, 0:H]),
                            in1=rec[:].unsqueeze(2).broadcast_to([P, HEADS, CH]),
                            op=OP.mult)
                    else:
                        nc.gpsimd.memset(comb[:], 0.0)
                    if use_bias_a[l]:
                        nc.vector.tensor_add(comb[:], comb[:], OPT[f"bias_a{l}"][:])
                    layer_norm_relu_resid(
                        comb, h_a, t,
                        OPT[f"lng_a{l}"] if use_ln_g[l][1] else None,
                        OPT[f"lnb_a{l}"] if use_ln_b[l][1] else None)

            # ---- output projections ----
            for t in range(PT):
                rows = min(P, PSH - t * P)
                hT = transpose_to_sbuf(h_p[:, t * H:(t + 1) * H], "hTo")
                om = psA.tile([P, H], f32, tag="T")
                nc.tensor.matmul(out=om[:], lhsT=hT[:], rhs=w_out_p[:],
                                 start=True, stop=True)
                ost = wp.tile([P, H], f32, tag="ost")
                if out_p_b:
                    nc.vector.tensor_add(ost[:], om[:], OPT["out_p_b"][:])
                else:
                    nc.scalar.activation(out=ost[:], in_=om[:], func=AF.Identity)
                nc.sync.dma_start(o_p[t * P:t * P + rows, :], ost[:rows, :])
            for t in range(AT):
                rows = min(P, ASH - t * P)
                hT = transpose_to_sbuf(h_a[:, t * H:(t + 1) * H], "hTo")
                om = psA.tile([P, H], f32, tag="T")
                nc.tensor.matmul(out=om[:], lhsT=hT[:], rhs=w_out_a[:],
                                 start=True, stop=True)
                ost = wp.tile([P, H], f32, tag="ost")
                if out_a_b:
                    nc.vector.tensor_add(ost[:], om[:], OPT["out_a_b"][:])
                else:
                    nc.scalar.activation(out=ost[:], in_=om[:], func=AF.Identity)
                nc.sync.dma_start(o_a[t * P:t * P + rows, :], ost[:rows, :])

    nc.compile()
    res = run_bass_kernel_spmd(nc, in_maps, list(range(NCORES)))
    out_author = np.concatenate([res.results[r]["o_a"] for r in range(NCORES)], 0)
    out_paper = np.concatenate([res.results[r]["o_p"] for r in range(NCORES)], 0)
    return out_author, out_paper


def kernel(**inputs):
    return _build_and_run(inputs)
